# revision 16
# baseline (speedup 1.0000x reference)
"""DGCNN (4x EdgeConv + final 1x1 conv) Bass/Tile kernel for Trainium2.

Sharding: pure data parallel — one point cloud (batch element) per
NeuronCore, conv weights replicated. B=8, N=2048, K=20.

On-chip layout is channels-first: activations live as [C, N] SBUF tiles so
that

  * kNN ranking keys come straight out of the tensor engine as
    [128 points, 2048 candidates] PSUM tiles: key(i,j) = x_i.x_j - |x_j|^2/2
    (row-constant terms don't change per-row ordering). The -|x_j|^2/2 term
    rides along as an extra contraction row (squared norms are stored as an
    extra row of the activation buffer) when C+1 <= 128, else as a separate
    K=1 accumulating matmul.
  * per-row exact top-20 runs on the vector engine via max8 / max_index /
    match_replace8 rounds (3 rounds = top-24 superset, first 20 in rank
    order are exactly jax.lax.top_k's set).
  * neighbor feature gathering is a free-dim gather shared across channels
    (gpsimd ap_gather); gather column order is j = 128*k + i so the
    required "wrapped" index layout is reachable with affine DMA views.
  * EdgeConv: h(i,k) = nbr@Wd + [x_i@(Wc-Wd) + b], with the center/bias
    term computed once per layer as t2 = (Wc-Wd).T @ X + b and added after
    the k-max (both commute with max over k); BN scale g is folded into the
    weights on the host; LeakyReLU is monotone so it's applied after k-max.
  * max over k=20 neighbors is a strided reduce straight out of PSUM.
"""

import sys

import numpy as np

for _p in ("/opt/trn_rl_repo", "/root/.axon_site/_ro/trn_rl_repo"):
    if _p not in sys.path:
        sys.path.insert(0, _p)

N = 2048
KNN = 20
NT = 16  # row tiles of 128 points
NCORES = 8

# cin: padded channel rows of the input buffer (gather needs %16);
# creal: real channels (xx row sits at row creal when cin allows C+1<=128)
LAYERS = [
    dict(cin=16, creal=3, cout=64, aug=True),
    dict(cin=64, creal=64, cout=64, aug=True),
    dict(cin=64, creal=64, cout=128, aug=True),
    dict(cin=128, creal=128, cout=256, aug=False),
]

_CACHE = {}
LAST_RESULTS = None  # BassKernelResults of the most recent run (for profiling)


def _build_program():
    import concourse.bass as bass  # noqa: F401
    import concourse.mybir as mybir
    import concourse.tile as tile
    from concourse import bacc
    from concourse.masks import make_identity
    from contextlib import ExitStack

    f32 = mybir.dt.float32
    u16 = mybir.dt.uint16
    i16 = mybir.dt.int16
    AF = mybir.ActivationFunctionType
    ALU = mybir.AluOpType

    nc = bacc.Bacc("TRN2", target_bir_lowering=False, debug=False,
                   enable_asserts=False)

    # ---- DRAM I/O ----
    x0_d = nc.dram_tensor("x0", [16, N], f32, kind="ExternalInput").ap()
    wd_d, wc_d, b_d = [], [], []
    for li, L in enumerate(LAYERS, start=1):
        wd_d.append(nc.dram_tensor(f"wd{li}", [L["cin"], L["cout"]], f32,
                                   kind="ExternalInput").ap())
        wc_d.append(nc.dram_tensor(f"wc{li}", [L["cin"], L["cout"]], f32,
                                   kind="ExternalInput").ap())
        bshape = [min(L["cout"], 128), L["cout"] // 128 if L["cout"] > 128 else 1]
        b_d.append(nc.dram_tensor(f"b{li}", bshape, f32,
                                  kind="ExternalInput").ap())
    w5_d = [nc.dram_tensor(f"w5{t}", [k, 1024], f32, kind="ExternalInput").ap()
            for t, k in (("a", 64), ("b", 64), ("c", 128), ("d", 128), ("e", 128))]
    b5_d = nc.dram_tensor("b5", [128, 8], f32, kind="ExternalInput").ap()
    out_d = nc.dram_tensor("out", [1024, N], f32, kind="ExternalOutput").ap()

    with tile.TileContext(nc) as tc, ExitStack() as ctx:
        wp = ctx.enter_context(tc.tile_pool(name="wp", bufs=1))
        xp = ctx.enter_context(tc.tile_pool(name="xp", bufs=1))
        sb = ctx.enter_context(tc.tile_pool(name="sb", bufs=2))
        psD = ctx.enter_context(tc.tile_pool(name="psD", bufs=4, space="PSUM"))
        psE = ctx.enter_context(tc.tile_pool(name="psE", bufs=4, space="PSUM"))
        dr = ctx.enter_context(tc.tile_pool(name="dr", bufs=2, space="DRAM"))

        def load(dram_ap, tag):
            t = wp.tile(list(dram_ap.shape), dram_ap.dtype, tag=tag)
            nc.sync.dma_start(out=t[:], in_=dram_ap)
            return t

        wd = [load(a, f"wd{i}") for i, a in enumerate(wd_d)]
        wc = [load(a, f"wc{i}") for i, a in enumerate(wc_d)]
        bb = [load(a, f"b{i}") for i, a in enumerate(b_d)]
        w5 = [load(a, f"w5{i}") for i, a in enumerate(w5_d)]
        b5 = load(b5_d, "b5")

        # persistent activation buffers (channels-first; one spare row for
        # the squared-norm "xx" row on aug layers)
        X0 = xp.tile([16, N], f32, tag="X0")
        nc.sync.dma_start(out=X0[:], in_=x0_d)
        X1 = xp.tile([65, N], f32, tag="X1")
        X2 = xp.tile([65, N], f32, tag="X2")
        X3 = xp.tile([128, N], f32, tag="X3")
        X4a = xp.tile([128, N], f32, tag="X4a")
        X4b = xp.tile([128, N], f32, tag="X4b")

        xxrow = xp.tile([1, N], f32, tag="xxrow")  # layer-4 xx (no spare row)
        scratch = xp.tile([128, N], f32, tag="scratch")

        ones_c = wp.tile([128, 1], f32, tag="ones_c")
        nc.vector.memset(ones_c[:], 1.0)
        neghalf = wp.tile([1, 128], f32, tag="neghalf")
        nc.vector.memset(neghalf[:], -0.5)
        identity = wp.tile([128, 128], f32, tag="identity")
        make_identity(nc, identity[:])

        xin = [X0, X1, X2, X3]
        xout = [[(X1, 0)], [(X2, 0)], [(X3, 0)], [(X4a, 0), (X4b, 0)]]

        for li, L in enumerate(LAYERS):
            cin, creal, cout, aug = L["cin"], L["creal"], L["cout"], L["aug"]
            Xi = xin[li]
            nct = len(xout[li])  # cout tiles

            # ---- xx[j] = sum_c Xi[c, j]^2 (into spare row / xxrow) ----
            nc.scalar.square(out=scratch[0:creal, :], in_=Xi[0:creal, :])
            # engine APs need 32-aligned start partitions; route through
            # xxrow (partition 0) and DMA into the spare row if unaligned
            direct = aug and creal % 32 == 0
            xx_dst = Xi[creal:creal + 1, :] if direct else xxrow[:]
            for q in range(4):
                pxx = psD.tile([128, 512], f32, tag="psD")
                nc.tensor.matmul(pxx[0:1, :], lhsT=ones_c[0:creal, 0:1],
                                 rhs=scratch[0:creal, q * 512:(q + 1) * 512],
                                 start=True, stop=True)
                nc.scalar.copy(out=xx_dst[:, q * 512:(q + 1) * 512],
                               in_=pxx[0:1, :])
            if aug and not direct:
                nc.sync.dma_start(out=Xi[creal:creal + 1, :], in_=xxrow[:])

            # ---- t2 = (Wc-Wd).T @ X + b, once per layer ----
            t2t = []
            for m in range(nct):
                t2m = sb.tile([128, N], f32, tag=f"t2{m}", bufs=1)
                t2t.append(t2m)
            for m in range(nct):
                mm = min(128, cout - m * 128)
                for q in range(4):
                    cs = q * 512
                    pt = psD.tile([128, 512], f32, tag="psD")
                    nc.tensor.matmul(pt[0:mm, :],
                                     lhsT=wc[li][0:cin, m * 128:m * 128 + mm],
                                     rhs=Xi[0:cin, cs:cs + 512],
                                     start=True, stop=True)
                    nc.scalar.activation(out=t2t[m][0:mm, cs:cs + 512],
                                         in_=pt[0:mm, :], func=AF.Identity,
                                         bias=bb[li][0:mm, m:m + 1], scale=1.0)

            def stage_front(r):
                rc = r * 128
                # ---- ranking keys: inner(i,j) - xx[j]/2 ----
                dist = sb.tile([128, N], f32, tag="dist", bufs=4)
                if aug:
                    la = sb.tile([creal + 1, 128], f32, tag="la")
                    nc.vector.memset(la[:], -0.5)
                    nc.scalar.copy(out=la[0:creal, :],
                                   in_=Xi[0:creal, rc:rc + 128])
                for q in range(4):
                    pd = psD.tile([128, 512], f32, tag="psD")
                    cs = q * 512
                    if aug:
                        nc.tensor.matmul(pd[:], lhsT=la[:],
                                         rhs=Xi[0:creal + 1, cs:cs + 512],
                                         start=True, stop=True)
                    else:
                        nc.tensor.matmul(pd[:], lhsT=Xi[0:creal, rc:rc + 128],
                                         rhs=Xi[0:creal, cs:cs + 512],
                                         start=True, stop=False)
                        nc.tensor.matmul(pd[:], lhsT=neghalf[:, 0:128],
                                         rhs=xxrow[:, cs:cs + 512],
                                         start=False, stop=True)
                    nc.scalar.copy(out=dist[:, cs:cs + 512], in_=pd[:])

                # ---- exact top-24 (>= top-20) per row ----
                idx = sb.tile([128, 24], u16, tag="idx", bufs=3)
                m8 = sb.tile([128, 8], f32, tag="m8")
                for rnd in range(3):
                    nc.vector.max(out=m8[:], in_=dist[:])
                    nc.vector.max_index(
                        out=idx[:, rnd * 8:rnd * 8 + 8],
                        in_max=m8[:], in_values=dist[:])
                    if rnd < 2:
                        nc.vector.match_replace(out=dist[:], in_to_replace=m8[:],
                                                in_values=dist[:],
                                                imm_value=-1e30)

                # ---- wrapped index layout for ap_gather, without scatter
                # DMAs: PE-transpose idx to [24 k, 128 i]. For k<16 the
                # wrapped layout of column order cA = 16*i + k IS the
                # transposed matrix; the k=16..19 tail wraps as the flat
                # affine view addr = 2048 + 32*p + s of the DRAM spill.
                idx_f = sb.tile([128, 24], f32, tag="idx_f")
                nc.vector.tensor_copy(idx_f[:], idx[:])
                pT = psD.tile([128, 512], f32, tag="psD")
                nc.tensor.transpose(pT[0:24, 0:128], idx_f[:], identity[:])
                idxT = sb.tile([24, 128], i16, tag="idxT", bufs=3)
                nc.vector.tensor_copy(idxT[:], pT[0:24, 0:128])
                scrT = dr.tile([24, 128], i16, tag="scrT", bufs=3)
                nc.sync.dma_start(out=scrT[:], in_=idxT[:])
                gidx = sb.tile([cin, 160], i16, tag="gidx", bufs=3)
                srcB = scrT[16:20, :].rearrange("a (b s) -> (a b) s", s=32)
                for g in range(cin // 16):
                    nc.sync.dma_start(out=gidx[g * 16:(g + 1) * 16, 0:128],
                                      in_=scrT[0:16, :])
                    nc.sync.dma_start(out=gidx[g * 16:(g + 1) * 16, 128:160],
                                      in_=srcB)
                return gidx

            def stage_back(r, gidx):
                rc = r * 128
                # ---- gather neighbor features (A: k<16, B: k=16..19) ----
                nbr = sb.tile([cin, KNN * 128], f32, tag="nbr", bufs=3)
                nc.gpsimd.ap_gather(out_ap=nbr[0:cin, 0:2048],
                                    in_ap=Xi[0:cin, 0:N],
                                    idxs_ap=gidx[0:cin, 0:128], channels=cin,
                                    num_elems=N, d=1, num_idxs=2048)
                nc.gpsimd.ap_gather(out_ap=nbr[0:cin, 2048:2560],
                                    in_ap=Xi[0:cin, 0:N],
                                    idxs_ap=gidx[0:cin, 128:160], channels=cin,
                                    num_elems=N, d=1, num_idxs=512)

                # ---- EdgeConv neighbor matmuls + piecewise k-max:
                # each 512-col A piece holds 32 points x 16 k (contiguous k),
                # so it reduces straight out of a 1-bank PSUM tile; the B
                # piece (k=16..19 for all 128 points) reduces separately and
                # merges with a small elementwise max.
                for m, (Xo, row0) in enumerate(xout[li]):
                    mm = min(128, cout - m * 128)
                    for q in range(4):
                        pq = psE.tile([128, 512], f32, tag="psE")
                        nc.tensor.matmul(
                            pq[0:mm, :],
                            lhsT=wd[li][0:cin, m * 128:m * 128 + mm],
                            rhs=nbr[0:cin, q * 512:(q + 1) * 512],
                            start=True, stop=True)
                        nc.vector.tensor_reduce(
                            out=Xo[row0:row0 + mm, rc + 32 * q:rc + 32 * q + 32],
                            in_=pq[0:mm, :].rearrange("c (i k) -> c i k", k=16),
                            axis=mybir.AxisListType.X, op=ALU.max)
                    pb = psE.tile([128, 512], f32, tag="psE")
                    nc.tensor.matmul(
                        pb[0:mm, :],
                        lhsT=wd[li][0:cin, m * 128:m * 128 + mm],
                        rhs=nbr[0:cin, 2048:2560],
                        start=True, stop=True)
                    # B column layout: 16*(i%32) + 4*k + (i//32)
                    tmpB = sb.tile([128, 128], f32, tag="tmpB")
                    nc.vector.tensor_reduce(
                        out=tmpB[0:mm, :].rearrange("c (i1 i0) -> c i1 i0",
                                                    i1=4),
                        in_=pb[0:mm, :].rearrange(
                            "c (i0 k i1) -> c i1 i0 k", k=4, i1=4),
                        axis=mybir.AxisListType.X, op=ALU.max)
                    nc.vector.tensor_max(Xo[row0:row0 + mm, rc:rc + 128],
                                         Xo[row0:row0 + mm, rc:rc + 128],
                                         tmpB[0:mm, :])

            # software pipeline: gather/conv for tile r-2 runs while top-k
            # chews tile r, so DVE never stalls on the conv chain
            PIPE = 3
            gq = {}
            for r in range(NT + PIPE):
                if r < NT:
                    gq[r] = stage_front(r)
                if r >= PIPE:
                    stage_back(r - PIPE, gq.pop(r - PIPE))

            # ---- epilogue: X = leaky(kmax + t2) (on gpsimd, DVE is busy) ----
            for m, (Xo, row0) in enumerate(xout[li]):
                mm = min(128, cout - m * 128)
                rows = Xo[row0:row0 + mm, :]
                nc.vector.tensor_add(rows, rows, t2t[m][0:mm, :])
                nc.vector.tensor_scalar_mul(scratch[0:mm, :], rows, 0.2)
                nc.vector.tensor_max(rows, rows, scratch[0:mm, :])

        # ---- final 1x1 conv: out = leaky(cat @ W5g + b5), channels-first ----
        ktiles = [(X1, 64, w5[0]), (X2, 64, w5[1]), (X3, 128, w5[2]),
                  (X4a, 128, w5[3]), (X4b, 128, w5[4])]
        for m in range(8):
            hst = sb.tile([128, N], f32, tag="dist", bufs=4)
            for q in range(4):
                cs = q * 512
                pF = psD.tile([128, 512], f32, tag="psD")
                for t, (Xt, kdim, Wt) in enumerate(ktiles):
                    nc.tensor.matmul(pF[:], lhsT=Wt[0:kdim, m * 128:(m + 1) * 128],
                                     rhs=Xt[0:kdim, cs:cs + 512],
                                     start=(t == 0), stop=(t == len(ktiles) - 1))
                nc.scalar.activation(out=hst[:, cs:cs + 512], in_=pF[:],
                                     func=AF.Identity,
                                     bias=b5[:, m:m + 1], scale=1.0)
            nc.vector.tensor_scalar_mul(scratch[:], hst[:], 0.2)
            nc.vector.tensor_max(hst[:], hst[:], scratch[:])
            nc.sync.dma_start(out=out_d[m * 128:(m + 1) * 128, :], in_=hst[:])

    nc.compile()
    return nc


def _get_program():
    if "nc" not in _CACHE:
        _CACHE["nc"] = _build_program()
    return _CACHE["nc"]


def make_in_maps(pts, W1, g1, b1, W2, g2, b2, W3, g3, b3, W4, g4, b4,
                 W5, g5, b5):
    """Host-side preprocessing: fold BN scale into weights, split the
    EdgeConv concat algebra, shard batch across cores."""
    f = np.float32
    shared = {}
    Ws = [(W1, g1, b1), (W2, g2, b2), (W3, g3, b3), (W4, g4, b4)]
    for li, ((W, g, b), L) in enumerate(zip(Ws, LAYERS), start=1):
        W = np.asarray(W, f) * np.asarray(g, f)[None, :]
        c2 = W.shape[0]
        c = c2 // 2
        wd = W[:c]                # multiplies (x_j - x_i)
        wcn = W[c:] - W[:c]       # multiplies x_i after folding the subtract
        cin = L["cin"]
        wd_p = np.zeros((cin, W.shape[1]), f)
        wc_p = np.zeros((cin, W.shape[1]), f)
        wd_p[:c] = wd
        wc_p[:c] = wcn
        shared[f"wd{li}"] = wd_p
        shared[f"wc{li}"] = wc_p
        b = np.asarray(b, f)
        if b.shape[0] > 128:
            b = b.reshape(-1, 128).T.copy()
        else:
            b = b.reshape(-1, 1)
        shared[f"b{li}"] = np.ascontiguousarray(b)

    W5g = np.asarray(W5, f) * np.asarray(g5, f)[None, :]
    for t, (lo, hi) in zip("abcde", ((0, 64), (64, 128), (128, 256),
                                     (256, 384), (384, 512))):
        shared[f"w5{t}"] = np.ascontiguousarray(W5g[lo:hi])
    shared["b5"] = np.ascontiguousarray(np.asarray(b5, f).reshape(8, 128).T)

    pts = np.asarray(pts, f)
    in_maps = []
    for core in range(NCORES):
        x0 = np.zeros((16, N), f)
        x0[0:3] = pts[core, :, 0:3].T
        in_maps.append({"x0": x0, **shared})
    return in_maps


def kernel(**inputs):
    global LAST_RESULTS
    from concourse.bass_utils import run_bass_kernel_spmd

    nc = _get_program()
    in_maps = make_in_maps(**inputs)
    trace = bool(_CACHE.get("trace"))
    res = run_bass_kernel_spmd(nc, in_maps, list(range(NCORES)), trace=trace)
    LAST_RESULTS = res
    return np.stack([r["out"] for r in res.results]).astype(np.float32)


# revision 17
# speedup vs baseline: 1.0069x; 1.0069x over previous
"""DGCNN (4x EdgeConv + final 1x1 conv) Bass/Tile kernel for Trainium2.

Sharding: pure data parallel — one point cloud (batch element) per
NeuronCore, conv weights replicated. B=8, N=2048, K=20.

On-chip layout is channels-first: activations live as [C, N] SBUF tiles so
that

  * kNN ranking keys come straight out of the tensor engine as
    [128 points, 2048 candidates] PSUM tiles: key(i,j) = x_i.x_j - |x_j|^2/2
    (row-constant terms don't change per-row ordering). The -|x_j|^2/2 term
    rides along as an extra contraction row (squared norms are stored as an
    extra row of the activation buffer) when C+1 <= 128, else as a separate
    K=1 accumulating matmul.
  * per-row exact top-20 runs on the vector engine via max8 / max_index /
    match_replace8 rounds (3 rounds = top-24 superset, first 20 in rank
    order are exactly jax.lax.top_k's set).
  * neighbor feature gathering is a free-dim gather shared across channels
    (gpsimd ap_gather); gather column order is j = 128*k + i so the
    required "wrapped" index layout is reachable with affine DMA views.
  * EdgeConv: h(i,k) = nbr@Wd + [x_i@(Wc-Wd) + b], with the center/bias
    term computed once per layer as t2 = (Wc-Wd).T @ X + b and added after
    the k-max (both commute with max over k); BN scale g is folded into the
    weights on the host; LeakyReLU is monotone so it's applied after k-max.
  * max over k=20 neighbors is a strided reduce straight out of PSUM.
"""

import sys

import numpy as np

for _p in ("/opt/trn_rl_repo", "/root/.axon_site/_ro/trn_rl_repo"):
    if _p not in sys.path:
        sys.path.insert(0, _p)

N = 2048
KNN = 20
NT = 16  # row tiles of 128 points
NCORES = 8

# cin: padded channel rows of the input buffer (gather needs %16);
# creal: real channels (xx row sits at row creal when cin allows C+1<=128)
LAYERS = [
    dict(cin=16, creal=3, cout=64, aug=True),
    dict(cin=64, creal=64, cout=64, aug=True),
    dict(cin=64, creal=64, cout=128, aug=True),
    dict(cin=128, creal=128, cout=256, aug=False),
]

_CACHE = {}
LAST_RESULTS = None  # BassKernelResults of the most recent run (for profiling)


def _build_program():
    import concourse.bass as bass  # noqa: F401
    import concourse.mybir as mybir
    import concourse.tile as tile
    from concourse import bacc
    from concourse.masks import make_identity
    from contextlib import ExitStack

    f32 = mybir.dt.float32
    u16 = mybir.dt.uint16
    i16 = mybir.dt.int16
    AF = mybir.ActivationFunctionType
    ALU = mybir.AluOpType

    nc = bacc.Bacc("TRN2", target_bir_lowering=False, debug=False,
                   enable_asserts=False)

    # ---- DRAM I/O ----
    x0_d = nc.dram_tensor("x0", [16, N], f32, kind="ExternalInput").ap()
    wd_d, wc_d, b_d = [], [], []
    for li, L in enumerate(LAYERS, start=1):
        wd_d.append(nc.dram_tensor(f"wd{li}", [L["cin"], L["cout"]], f32,
                                   kind="ExternalInput").ap())
        wc_d.append(nc.dram_tensor(f"wc{li}", [L["cin"], L["cout"]], f32,
                                   kind="ExternalInput").ap())
        bshape = [min(L["cout"], 128), L["cout"] // 128 if L["cout"] > 128 else 1]
        b_d.append(nc.dram_tensor(f"b{li}", bshape, f32,
                                  kind="ExternalInput").ap())
    w5_d = [nc.dram_tensor(f"w5{t}", [k, 1024], f32, kind="ExternalInput").ap()
            for t, k in (("a", 64), ("b", 64), ("c", 128), ("d", 128), ("e", 128))]
    b5_d = nc.dram_tensor("b5", [128, 8], f32, kind="ExternalInput").ap()
    out_d = nc.dram_tensor("out", [1024, N], f32, kind="ExternalOutput").ap()

    with tile.TileContext(nc) as tc, ExitStack() as ctx:
        wp = ctx.enter_context(tc.tile_pool(name="wp", bufs=1))
        xp = ctx.enter_context(tc.tile_pool(name="xp", bufs=1))
        sb = ctx.enter_context(tc.tile_pool(name="sb", bufs=2))
        psD = ctx.enter_context(tc.tile_pool(name="psD", bufs=4, space="PSUM"))
        psE = ctx.enter_context(tc.tile_pool(name="psE", bufs=4, space="PSUM"))
        dr = ctx.enter_context(tc.tile_pool(name="dr", bufs=2, space="DRAM"))

        def load(dram_ap, tag):
            t = wp.tile(list(dram_ap.shape), dram_ap.dtype, tag=tag)
            nc.sync.dma_start(out=t[:], in_=dram_ap)
            return t

        wd = [load(a, f"wd{i}") for i, a in enumerate(wd_d)]
        wc = [load(a, f"wc{i}") for i, a in enumerate(wc_d)]
        bb = [load(a, f"b{i}") for i, a in enumerate(b_d)]
        w5 = [load(a, f"w5{i}") for i, a in enumerate(w5_d)]
        b5 = load(b5_d, "b5")

        # persistent activation buffers (channels-first; one spare row for
        # the squared-norm "xx" row on aug layers)
        X0 = xp.tile([16, N], f32, tag="X0")
        nc.sync.dma_start(out=X0[:], in_=x0_d)
        X1 = xp.tile([65, N], f32, tag="X1")
        X2 = xp.tile([65, N], f32, tag="X2")
        X3 = xp.tile([128, N], f32, tag="X3")
        X4a = xp.tile([128, N], f32, tag="X4a")
        X4b = xp.tile([128, N], f32, tag="X4b")

        xxrow = xp.tile([1, N], f32, tag="xxrow")  # layer-4 xx (no spare row)
        scratch = xp.tile([128, N], f32, tag="scratch")

        ones_c = wp.tile([128, 1], f32, tag="ones_c")
        nc.vector.memset(ones_c[:], 1.0)
        neghalf = wp.tile([1, 128], f32, tag="neghalf")
        nc.vector.memset(neghalf[:], -0.5)
        identity = wp.tile([128, 128], f32, tag="identity")
        make_identity(nc, identity[:])

        xin = [X0, X1, X2, X3]
        xout = [[(X1, 0)], [(X2, 0)], [(X3, 0)], [(X4a, 0), (X4b, 0)]]

        for li, L in enumerate(LAYERS):
            cin, creal, cout, aug = L["cin"], L["creal"], L["cout"], L["aug"]
            Xi = xin[li]
            nct = len(xout[li])  # cout tiles

            # ---- xx[j] = sum_c Xi[c, j]^2 (into spare row / xxrow) ----
            nc.scalar.square(out=scratch[0:creal, :], in_=Xi[0:creal, :])
            # engine APs need 32-aligned start partitions; route through
            # xxrow (partition 0) and DMA into the spare row if unaligned
            direct = aug and creal % 32 == 0
            xx_dst = Xi[creal:creal + 1, :] if direct else xxrow[:]
            for q in range(4):
                pxx = psD.tile([128, 512], f32, tag="psD")
                nc.tensor.matmul(pxx[0:1, :], lhsT=ones_c[0:creal, 0:1],
                                 rhs=scratch[0:creal, q * 512:(q + 1) * 512],
                                 start=True, stop=True)
                nc.scalar.copy(out=xx_dst[:, q * 512:(q + 1) * 512],
                               in_=pxx[0:1, :])
            if aug and not direct:
                nc.sync.dma_start(out=Xi[creal:creal + 1, :], in_=xxrow[:])

            # ---- t2 = (Wc-Wd).T @ X + b, once per layer ----
            t2t = []
            for m in range(nct):
                t2m = sb.tile([128, N], f32, tag=f"t2{m}", bufs=1)
                t2t.append(t2m)
            for m in range(nct):
                mm = min(128, cout - m * 128)
                for q in range(4):
                    cs = q * 512
                    pt = psD.tile([128, 512], f32, tag="psD")
                    nc.tensor.matmul(pt[0:mm, :],
                                     lhsT=wc[li][0:cin, m * 128:m * 128 + mm],
                                     rhs=Xi[0:cin, cs:cs + 512],
                                     start=True, stop=True)
                    nc.scalar.activation(out=t2t[m][0:mm, cs:cs + 512],
                                         in_=pt[0:mm, :], func=AF.Identity,
                                         bias=bb[li][0:mm, m:m + 1], scale=1.0)

            def stage_front(r):
                rc = r * 128
                # ---- ranking keys: inner(i,j) - xx[j]/2 ----
                dist = sb.tile([128, N], f32, tag="dist", bufs=4)
                if aug:
                    la = sb.tile([creal + 1, 128], f32, tag="la")
                    nc.gpsimd.memset(la[:], -0.5)
                    nc.scalar.copy(out=la[0:creal, :],
                                   in_=Xi[0:creal, rc:rc + 128])
                for q in range(4):
                    pd = psD.tile([128, 512], f32, tag="psD")
                    cs = q * 512
                    if aug:
                        nc.tensor.matmul(pd[:], lhsT=la[:],
                                         rhs=Xi[0:creal + 1, cs:cs + 512],
                                         start=True, stop=True)
                    else:
                        nc.tensor.matmul(pd[:], lhsT=Xi[0:creal, rc:rc + 128],
                                         rhs=Xi[0:creal, cs:cs + 512],
                                         start=True, stop=False)
                        nc.tensor.matmul(pd[:], lhsT=neghalf[:, 0:128],
                                         rhs=xxrow[:, cs:cs + 512],
                                         start=False, stop=True)
                    nc.scalar.copy(out=dist[:, cs:cs + 512], in_=pd[:])

                # ---- exact top-24 (>= top-20) per row ----
                idx = sb.tile([128, 24], u16, tag="idx", bufs=3)
                m8 = sb.tile([128, 8], f32, tag="m8")
                for rnd in range(3):
                    nc.vector.max(out=m8[:], in_=dist[:])
                    nc.vector.max_index(
                        out=idx[:, rnd * 8:rnd * 8 + 8],
                        in_max=m8[:], in_values=dist[:])
                    if rnd < 2:
                        nc.vector.match_replace(out=dist[:], in_to_replace=m8[:],
                                                in_values=dist[:],
                                                imm_value=-1e30)

                # ---- wrapped index layout for ap_gather, without scatter
                # DMAs: PE-transpose idx to [24 k, 128 i]. For k<16 the
                # wrapped layout of column order cA = 16*i + k IS the
                # transposed matrix; the k=16..19 tail wraps as the flat
                # affine view addr = 2048 + 32*p + s of the DRAM spill.
                idx_f = sb.tile([128, 24], f32, tag="idx_f")
                nc.vector.tensor_copy(idx_f[:], idx[:])
                pT = psD.tile([128, 512], f32, tag="psD")
                nc.tensor.transpose(pT[0:24, 0:128], idx_f[:], identity[:])
                idxT = sb.tile([24, 128], i16, tag="idxT", bufs=3)
                nc.vector.tensor_copy(idxT[:], pT[0:24, 0:128])
                scrT = dr.tile([24, 128], i16, tag="scrT", bufs=3)
                nc.sync.dma_start(out=scrT[:], in_=idxT[:])
                gidx = sb.tile([cin, 160], i16, tag="gidx", bufs=3)
                srcB = scrT[16:20, :].rearrange("a (b s) -> (a b) s", s=32)
                for g in range(cin // 16):
                    nc.sync.dma_start(out=gidx[g * 16:(g + 1) * 16, 0:128],
                                      in_=scrT[0:16, :])
                    nc.sync.dma_start(out=gidx[g * 16:(g + 1) * 16, 128:160],
                                      in_=srcB)
                return gidx

            def stage_back(r, gidx):
                rc = r * 128
                # ---- gather neighbor features (A: k<16, B: k=16..19) ----
                nbr = sb.tile([cin, KNN * 128], f32, tag="nbr", bufs=3)
                nc.gpsimd.ap_gather(out_ap=nbr[0:cin, 0:2048],
                                    in_ap=Xi[0:cin, 0:N],
                                    idxs_ap=gidx[0:cin, 0:128], channels=cin,
                                    num_elems=N, d=1, num_idxs=2048)
                nc.gpsimd.ap_gather(out_ap=nbr[0:cin, 2048:2560],
                                    in_ap=Xi[0:cin, 0:N],
                                    idxs_ap=gidx[0:cin, 128:160], channels=cin,
                                    num_elems=N, d=1, num_idxs=512)

                # ---- EdgeConv neighbor matmuls + piecewise k-max:
                # each 512-col A piece holds 32 points x 16 k (contiguous k),
                # so it reduces straight out of a 1-bank PSUM tile; the B
                # piece (k=16..19 for all 128 points) reduces separately and
                # merges with a small elementwise max.
                for m, (Xo, row0) in enumerate(xout[li]):
                    mm = min(128, cout - m * 128)
                    for q in range(4):
                        pq = psE.tile([128, 512], f32, tag="psE")
                        nc.tensor.matmul(
                            pq[0:mm, :],
                            lhsT=wd[li][0:cin, m * 128:m * 128 + mm],
                            rhs=nbr[0:cin, q * 512:(q + 1) * 512],
                            start=True, stop=True)
                        nc.vector.tensor_reduce(
                            out=Xo[row0:row0 + mm, rc + 32 * q:rc + 32 * q + 32],
                            in_=pq[0:mm, :].rearrange("c (i k) -> c i k", k=16),
                            axis=mybir.AxisListType.X, op=ALU.max)
                    pb = psE.tile([128, 512], f32, tag="psE")
                    nc.tensor.matmul(
                        pb[0:mm, :],
                        lhsT=wd[li][0:cin, m * 128:m * 128 + mm],
                        rhs=nbr[0:cin, 2048:2560],
                        start=True, stop=True)
                    # B column layout: 16*(i%32) + 4*k + (i//32)
                    tmpB = sb.tile([128, 128], f32, tag="tmpB")
                    nc.vector.tensor_reduce(
                        out=tmpB[0:mm, :].rearrange("c (i1 i0) -> c i1 i0",
                                                    i1=4),
                        in_=pb[0:mm, :].rearrange(
                            "c (i0 k i1) -> c i1 i0 k", k=4, i1=4),
                        axis=mybir.AxisListType.X, op=ALU.max)
                    nc.vector.tensor_max(Xo[row0:row0 + mm, rc:rc + 128],
                                         Xo[row0:row0 + mm, rc:rc + 128],
                                         tmpB[0:mm, :])

            # software pipeline: gather/conv for tile r-2 runs while top-k
            # chews tile r, so DVE never stalls on the conv chain
            PIPE = 3
            gq = {}
            for r in range(NT + PIPE):
                if r < NT:
                    gq[r] = stage_front(r)
                if r >= PIPE:
                    stage_back(r - PIPE, gq.pop(r - PIPE))

            # ---- epilogue: X = leaky(kmax + t2) (on gpsimd, DVE is busy) ----
            for m, (Xo, row0) in enumerate(xout[li]):
                mm = min(128, cout - m * 128)
                rows = Xo[row0:row0 + mm, :]
                nc.vector.tensor_add(rows, rows, t2t[m][0:mm, :])
                nc.vector.tensor_scalar_mul(scratch[0:mm, :], rows, 0.2)
                nc.vector.tensor_max(rows, rows, scratch[0:mm, :])

        # ---- final 1x1 conv: out = leaky(cat @ W5g + b5), channels-first ----
        ktiles = [(X1, 64, w5[0]), (X2, 64, w5[1]), (X3, 128, w5[2]),
                  (X4a, 128, w5[3]), (X4b, 128, w5[4])]
        for m in range(8):
            hst = sb.tile([128, N], f32, tag="dist", bufs=4)
            for q in range(4):
                cs = q * 512
                pF = psD.tile([128, 512], f32, tag="psD")
                for t, (Xt, kdim, Wt) in enumerate(ktiles):
                    nc.tensor.matmul(pF[:], lhsT=Wt[0:kdim, m * 128:(m + 1) * 128],
                                     rhs=Xt[0:kdim, cs:cs + 512],
                                     start=(t == 0), stop=(t == len(ktiles) - 1))
                nc.scalar.activation(out=hst[:, cs:cs + 512], in_=pF[:],
                                     func=AF.Identity,
                                     bias=b5[:, m:m + 1], scale=1.0)
            nc.vector.tensor_scalar_mul(scratch[:], hst[:], 0.2)
            nc.vector.tensor_max(hst[:], hst[:], scratch[:])
            nc.sync.dma_start(out=out_d[m * 128:(m + 1) * 128, :], in_=hst[:])

    nc.compile()
    return nc


def _get_program():
    if "nc" not in _CACHE:
        _CACHE["nc"] = _build_program()
    return _CACHE["nc"]


def make_in_maps(pts, W1, g1, b1, W2, g2, b2, W3, g3, b3, W4, g4, b4,
                 W5, g5, b5):
    """Host-side preprocessing: fold BN scale into weights, split the
    EdgeConv concat algebra, shard batch across cores."""
    f = np.float32
    shared = {}
    Ws = [(W1, g1, b1), (W2, g2, b2), (W3, g3, b3), (W4, g4, b4)]
    for li, ((W, g, b), L) in enumerate(zip(Ws, LAYERS), start=1):
        W = np.asarray(W, f) * np.asarray(g, f)[None, :]
        c2 = W.shape[0]
        c = c2 // 2
        wd = W[:c]                # multiplies (x_j - x_i)
        wcn = W[c:] - W[:c]       # multiplies x_i after folding the subtract
        cin = L["cin"]
        wd_p = np.zeros((cin, W.shape[1]), f)
        wc_p = np.zeros((cin, W.shape[1]), f)
        wd_p[:c] = wd
        wc_p[:c] = wcn
        shared[f"wd{li}"] = wd_p
        shared[f"wc{li}"] = wc_p
        b = np.asarray(b, f)
        if b.shape[0] > 128:
            b = b.reshape(-1, 128).T.copy()
        else:
            b = b.reshape(-1, 1)
        shared[f"b{li}"] = np.ascontiguousarray(b)

    W5g = np.asarray(W5, f) * np.asarray(g5, f)[None, :]
    for t, (lo, hi) in zip("abcde", ((0, 64), (64, 128), (128, 256),
                                     (256, 384), (384, 512))):
        shared[f"w5{t}"] = np.ascontiguousarray(W5g[lo:hi])
    shared["b5"] = np.ascontiguousarray(np.asarray(b5, f).reshape(8, 128).T)

    pts = np.asarray(pts, f)
    in_maps = []
    for core in range(NCORES):
        x0 = np.zeros((16, N), f)
        x0[0:3] = pts[core, :, 0:3].T
        in_maps.append({"x0": x0, **shared})
    return in_maps


def kernel(**inputs):
    global LAST_RESULTS
    from concourse.bass_utils import run_bass_kernel_spmd

    nc = _get_program()
    in_maps = make_in_maps(**inputs)
    trace = bool(_CACHE.get("trace"))
    res = run_bass_kernel_spmd(nc, in_maps, list(range(NCORES)), trace=trace)
    LAST_RESULTS = res
    return np.stack([r["out"] for r in res.results]).astype(np.float32)


# revision 18
# speedup vs baseline: 1.1951x; 1.1869x over previous
"""DGCNN (4x EdgeConv + final 1x1 conv) Bass/Tile kernel for Trainium2.

Sharding: pure data parallel — one point cloud (batch element) per
NeuronCore, conv weights replicated. B=8, N=2048, K=20.

On-chip layout is channels-first: activations live as [C, N] SBUF tiles so
that

  * kNN ranking keys come straight out of the tensor engine as
    [128 points, 2048 candidates] PSUM tiles: key(i,j) = x_i.x_j - |x_j|^2/2
    (row-constant terms don't change per-row ordering). The -|x_j|^2/2 term
    rides along as an extra contraction row (squared norms are stored as an
    extra row of the activation buffer) when C+1 <= 128, else as a separate
    K=1 accumulating matmul.
  * per-row exact top-20 runs on the vector engine via max8 / max_index /
    match_replace8 rounds (3 rounds = top-24 superset, first 20 in rank
    order are exactly jax.lax.top_k's set).
  * neighbor feature gathering is a free-dim gather shared across channels
    (gpsimd ap_gather); gather column order is j = 128*k + i so the
    required "wrapped" index layout is reachable with affine DMA views.
  * EdgeConv: h(i,k) = nbr@Wd + [x_i@(Wc-Wd) + b], with the center/bias
    term computed once per layer as t2 = (Wc-Wd).T @ X + b and added after
    the k-max (both commute with max over k); BN scale g is folded into the
    weights on the host; LeakyReLU is monotone so it's applied after k-max.
  * max over k=20 neighbors is a strided reduce straight out of PSUM.
"""

import sys

import numpy as np

for _p in ("/opt/trn_rl_repo", "/root/.axon_site/_ro/trn_rl_repo"):
    if _p not in sys.path:
        sys.path.insert(0, _p)

N = 2048
KNN = 20
NT = 16  # row tiles of 128 points
NCORES = 8

# cin: padded channel rows of the input buffer (gather needs %16);
# creal: real channels (xx row sits at row creal when cin allows C+1<=128)
LAYERS = [
    dict(cin=16, creal=3, cout=64, aug=True),
    dict(cin=64, creal=64, cout=64, aug=True),
    dict(cin=64, creal=64, cout=128, aug=True),
    dict(cin=128, creal=128, cout=256, aug=False),
]

_CACHE = {}
LAST_RESULTS = None  # BassKernelResults of the most recent run (for profiling)


def _build_program():
    import concourse.bass as bass  # noqa: F401
    import concourse.mybir as mybir
    import concourse.tile as tile
    from concourse import bacc
    from concourse.masks import make_identity
    from contextlib import ExitStack

    f32 = mybir.dt.float32
    u16 = mybir.dt.uint16
    i16 = mybir.dt.int16
    AF = mybir.ActivationFunctionType
    ALU = mybir.AluOpType

    nc = bacc.Bacc("TRN2", target_bir_lowering=False, debug=False,
                   enable_asserts=False)

    # ---- DRAM I/O ----
    x0_d = nc.dram_tensor("x0", [16, N], f32, kind="ExternalInput").ap()
    wd_d, wc_d, b_d = [], [], []
    for li, L in enumerate(LAYERS, start=1):
        wd_d.append(nc.dram_tensor(f"wd{li}", [L["cin"], L["cout"]], f32,
                                   kind="ExternalInput").ap())
        wc_d.append(nc.dram_tensor(f"wc{li}", [L["cin"], L["cout"]], f32,
                                   kind="ExternalInput").ap())
        bshape = [min(L["cout"], 128), L["cout"] // 128 if L["cout"] > 128 else 1]
        b_d.append(nc.dram_tensor(f"b{li}", bshape, f32,
                                  kind="ExternalInput").ap())
    w5_d = [nc.dram_tensor(f"w5{t}", [k, 1024], f32, kind="ExternalInput").ap()
            for t, k in (("a", 64), ("b", 64), ("c", 128), ("d", 128), ("e", 128))]
    b5_d = nc.dram_tensor("b5", [128, 8], f32, kind="ExternalInput").ap()
    out_d = nc.dram_tensor("out", [1024, N], f32, kind="ExternalOutput").ap()

    with tile.TileContext(nc) as tc, ExitStack() as ctx:
        wp = ctx.enter_context(tc.tile_pool(name="wp", bufs=1))
        xp = ctx.enter_context(tc.tile_pool(name="xp", bufs=1))
        sb = ctx.enter_context(tc.tile_pool(name="sb", bufs=2))
        psD = ctx.enter_context(tc.tile_pool(name="psD", bufs=6, space="PSUM"))
        dr = ctx.enter_context(tc.tile_pool(name="dr", bufs=2, space="DRAM"))

        def load(dram_ap, tag):
            t = wp.tile(list(dram_ap.shape), dram_ap.dtype, tag=tag)
            nc.sync.dma_start(out=t[:], in_=dram_ap)
            return t

        wd = [load(a, f"wd{i}") for i, a in enumerate(wd_d)]
        wc = [load(a, f"wc{i}") for i, a in enumerate(wc_d)]
        bb = [load(a, f"b{i}") for i, a in enumerate(b_d)]
        w5 = [load(a, f"w5{i}") for i, a in enumerate(w5_d)]
        b5 = load(b5_d, "b5")

        # persistent activation buffers (channels-first; one spare row for
        # the squared-norm "xx" row on aug layers)
        X0 = xp.tile([16, N], f32, tag="X0")
        nc.sync.dma_start(out=X0[:], in_=x0_d)
        X1 = xp.tile([65, N], f32, tag="X1")
        X2 = xp.tile([65, N], f32, tag="X2")
        X3 = xp.tile([128, N], f32, tag="X3")
        X4a = xp.tile([128, N], f32, tag="X4a")
        X4b = xp.tile([128, N], f32, tag="X4b")

        xxrow = xp.tile([1, N], f32, tag="xxrow")  # layer-4 xx (no spare row)
        scratch = xp.tile([128, N], f32, tag="scratch")

        ones_c = wp.tile([128, 1], f32, tag="ones_c")
        nc.vector.memset(ones_c[:], 1.0)
        neghalf = wp.tile([1, 128], f32, tag="neghalf")
        nc.vector.memset(neghalf[:], -0.5)
        identity = wp.tile([128, 128], f32, tag="identity")
        make_identity(nc, identity[:])

        xin = [X0, X1, X2, X3]
        xout = [[(X1, 0)], [(X2, 0)], [(X3, 0)], [(X4a, 0), (X4b, 0)]]

        for li, L in enumerate(LAYERS):
            cin, creal, cout, aug = L["cin"], L["creal"], L["cout"], L["aug"]
            Xi = xin[li]
            nct = len(xout[li])  # cout tiles

            # ---- xx[j] = sum_c Xi[c, j]^2 (into spare row / xxrow) ----
            nc.scalar.square(out=scratch[0:creal, :], in_=Xi[0:creal, :])
            # engine APs need 32-aligned start partitions; route through
            # xxrow (partition 0) and DMA into the spare row if unaligned
            direct = aug and creal % 32 == 0
            xx_dst = Xi[creal:creal + 1, :] if direct else xxrow[:]
            for q in range(4):
                pxx = psD.tile([128, 512], f32, tag="psD")
                nc.tensor.matmul(pxx[0:1, :], lhsT=ones_c[0:creal, 0:1],
                                 rhs=scratch[0:creal, q * 512:(q + 1) * 512],
                                 start=True, stop=True)
                nc.scalar.copy(out=xx_dst[:, q * 512:(q + 1) * 512],
                               in_=pxx[0:1, :])
            if aug and not direct:
                nc.sync.dma_start(out=Xi[creal:creal + 1, :], in_=xxrow[:])

            # ---- t2 = (Wc-Wd).T @ X + b, once per layer ----
            t2t = []
            for m in range(nct):
                t2m = sb.tile([128, N], f32, tag=f"t2{m}", bufs=1)
                t2t.append(t2m)
            for m in range(nct):
                mm = min(128, cout - m * 128)
                for q in range(4):
                    cs = q * 512
                    pt = psD.tile([128, 512], f32, tag="psD")
                    nc.tensor.matmul(pt[0:mm, :],
                                     lhsT=wc[li][0:cin, m * 128:m * 128 + mm],
                                     rhs=Xi[0:cin, cs:cs + 512],
                                     start=True, stop=True)
                    nc.scalar.activation(out=t2t[m][0:mm, cs:cs + 512],
                                         in_=pt[0:mm, :], func=AF.Identity,
                                         bias=bb[li][0:mm, m:m + 1], scale=1.0)

            def stage_front(r):
                rc = r * 128
                # ---- ranking keys: inner(i,j) - xx[j]/2 ----
                dist = sb.tile([128, N], f32, tag="dist", bufs=4)
                if aug:
                    la = sb.tile([creal + 1, 128], f32, tag="la")
                    nc.gpsimd.memset(la[:], -0.5)
                    nc.scalar.copy(out=la[0:creal, :],
                                   in_=Xi[0:creal, rc:rc + 128])
                for q in range(4):
                    pd = psD.tile([128, 512], f32, tag="psD")
                    cs = q * 512
                    if aug:
                        nc.tensor.matmul(pd[:], lhsT=la[:],
                                         rhs=Xi[0:creal + 1, cs:cs + 512],
                                         start=True, stop=True)
                    else:
                        nc.tensor.matmul(pd[:], lhsT=Xi[0:creal, rc:rc + 128],
                                         rhs=Xi[0:creal, cs:cs + 512],
                                         start=True, stop=False)
                        nc.tensor.matmul(pd[:], lhsT=neghalf[:, 0:128],
                                         rhs=xxrow[:, cs:cs + 512],
                                         start=False, stop=True)
                    nc.scalar.copy(out=dist[:, cs:cs + 512], in_=pd[:])

                # ---- exact top-24 (>= top-20) per row ----
                idx = sb.tile([128, 24], u16, tag="idx", bufs=3)
                m8 = sb.tile([128, 8], f32, tag="m8")
                for rnd in range(3):
                    nc.vector.max(out=m8[:], in_=dist[:])
                    nc.vector.max_index(
                        out=idx[:, rnd * 8:rnd * 8 + 8],
                        in_max=m8[:], in_values=dist[:])
                    if rnd < 2:
                        nc.vector.match_replace(out=dist[:], in_to_replace=m8[:],
                                                in_values=dist[:],
                                                imm_value=-1e30)

                # ---- wrapped index layout for ap_gather, without scatter
                # DMAs: PE-transpose idx to [24 k, 128 i]. For k<16 the
                # wrapped layout of column order cA = 16*i + k IS the
                # transposed matrix; the k=16..19 tail wraps as the flat
                # affine view addr = 2048 + 32*p + s of the DRAM spill.
                idx_f = sb.tile([128, 24], f32, tag="idx_f")
                nc.vector.tensor_copy(idx_f[:], idx[:])
                pT = psD.tile([128, 512], f32, tag="psD")
                nc.tensor.transpose(pT[0:24, 0:128], idx_f[:], identity[:])
                idxT = sb.tile([24, 128], i16, tag="idxT", bufs=3)
                nc.vector.tensor_copy(idxT[:], pT[0:24, 0:128])
                scrT = dr.tile([24, 128], i16, tag="scrT", bufs=3)
                nc.sync.dma_start(out=scrT[:], in_=idxT[:])
                gidx = sb.tile([cin, 160], i16, tag="gidx", bufs=3)
                srcB = scrT[16:20, :].rearrange("a (b s) -> (a b) s", s=32)
                for g in range(cin // 16):
                    nc.sync.dma_start(out=gidx[g * 16:(g + 1) * 16, 0:128],
                                      in_=scrT[0:16, :])
                    nc.sync.dma_start(out=gidx[g * 16:(g + 1) * 16, 128:160],
                                      in_=srcB)
                return gidx

            def stage_back(r, gidx):
                rc = r * 128
                # ---- gather neighbor features (A: k<16, B: k=16..19) ----
                nbr = sb.tile([cin, KNN * 128], f32, tag="nbr")
                nc.gpsimd.ap_gather(out_ap=nbr[0:cin, 0:2048],
                                    in_ap=Xi[0:cin, 0:N],
                                    idxs_ap=gidx[0:cin, 0:128], channels=cin,
                                    num_elems=N, d=1, num_idxs=2048)
                nc.gpsimd.ap_gather(out_ap=nbr[0:cin, 2048:2560],
                                    in_ap=Xi[0:cin, 0:N],
                                    idxs_ap=gidx[0:cin, 128:160], channels=cin,
                                    num_elems=N, d=1, num_idxs=512)

                # ---- EdgeConv neighbor matmuls + piecewise k-max:
                # each 512-col A piece holds 32 points x 16 k (contiguous k),
                # so it reduces straight out of a 1-bank PSUM tile; the B
                # piece (k=16..19 for all 128 points) reduces separately and
                # merges with a small elementwise max.
                for m, (Xo, row0) in enumerate(xout[li]):
                    mm = min(128, cout - m * 128)
                    for q in range(4):
                        pq = psD.tile([128, 512], f32, tag="psD")
                        nc.tensor.matmul(
                            pq[0:mm, :],
                            lhsT=wd[li][0:cin, m * 128:m * 128 + mm],
                            rhs=nbr[0:cin, q * 512:(q + 1) * 512],
                            start=True, stop=True)
                        nc.vector.tensor_reduce(
                            out=Xo[row0:row0 + mm, rc + 32 * q:rc + 32 * q + 32],
                            in_=pq[0:mm, :].rearrange("c (i k) -> c i k", k=16),
                            axis=mybir.AxisListType.X, op=ALU.max)
                    pb = psD.tile([128, 512], f32, tag="psD")
                    nc.tensor.matmul(
                        pb[0:mm, :],
                        lhsT=wd[li][0:cin, m * 128:m * 128 + mm],
                        rhs=nbr[0:cin, 2048:2560],
                        start=True, stop=True)
                    # B column layout: 16*(i%32) + 4*k + (i//32)
                    tmpB = sb.tile([128, 128], f32, tag="tmpB")
                    nc.vector.tensor_reduce(
                        out=tmpB[0:mm, :].rearrange("c (i1 i0) -> c i1 i0",
                                                    i1=4),
                        in_=pb[0:mm, :].rearrange(
                            "c (i0 k i1) -> c i1 i0 k", k=4, i1=4),
                        axis=mybir.AxisListType.X, op=ALU.max)
                    nc.vector.tensor_max(Xo[row0:row0 + mm, rc:rc + 128],
                                         Xo[row0:row0 + mm, rc:rc + 128],
                                         tmpB[0:mm, :])

            # software pipeline: gather/conv for tile r-2 runs while top-k
            # chews tile r, so DVE never stalls on the conv chain
            PIPE = 3
            gq = {}
            for r in range(NT + PIPE):
                if r < NT:
                    gq[r] = stage_front(r)
                if r >= PIPE:
                    stage_back(r - PIPE, gq.pop(r - PIPE))

            # ---- epilogue: X = leaky(kmax + t2) (on gpsimd, DVE is busy) ----
            for m, (Xo, row0) in enumerate(xout[li]):
                mm = min(128, cout - m * 128)
                rows = Xo[row0:row0 + mm, :]
                nc.vector.tensor_add(rows, rows, t2t[m][0:mm, :])
                nc.vector.tensor_scalar_mul(scratch[0:mm, :], rows, 0.2)
                nc.vector.tensor_max(rows, rows, scratch[0:mm, :])

        # ---- final 1x1 conv: out = leaky(cat @ W5g + b5), channels-first ----
        ktiles = [(X1, 64, w5[0]), (X2, 64, w5[1]), (X3, 128, w5[2]),
                  (X4a, 128, w5[3]), (X4b, 128, w5[4])]
        for m in range(8):
            hst = sb.tile([128, N], f32, tag="dist", bufs=4)
            for q in range(4):
                cs = q * 512
                pF = psD.tile([128, 512], f32, tag="psD")
                for t, (Xt, kdim, Wt) in enumerate(ktiles):
                    nc.tensor.matmul(pF[:], lhsT=Wt[0:kdim, m * 128:(m + 1) * 128],
                                     rhs=Xt[0:kdim, cs:cs + 512],
                                     start=(t == 0), stop=(t == len(ktiles) - 1))
                nc.scalar.activation(out=hst[:, cs:cs + 512], in_=pF[:],
                                     func=AF.Identity,
                                     bias=b5[:, m:m + 1], scale=1.0)
            nc.vector.tensor_scalar_mul(scratch[:], hst[:], 0.2)
            nc.vector.tensor_max(hst[:], hst[:], scratch[:])
            nc.sync.dma_start(out=out_d[m * 128:(m + 1) * 128, :], in_=hst[:])

    nc.compile()
    return nc


def _get_program():
    if "nc" not in _CACHE:
        _CACHE["nc"] = _build_program()
    return _CACHE["nc"]


def make_in_maps(pts, W1, g1, b1, W2, g2, b2, W3, g3, b3, W4, g4, b4,
                 W5, g5, b5):
    """Host-side preprocessing: fold BN scale into weights, split the
    EdgeConv concat algebra, shard batch across cores."""
    f = np.float32
    shared = {}
    Ws = [(W1, g1, b1), (W2, g2, b2), (W3, g3, b3), (W4, g4, b4)]
    for li, ((W, g, b), L) in enumerate(zip(Ws, LAYERS), start=1):
        W = np.asarray(W, f) * np.asarray(g, f)[None, :]
        c2 = W.shape[0]
        c = c2 // 2
        wd = W[:c]                # multiplies (x_j - x_i)
        wcn = W[c:] - W[:c]       # multiplies x_i after folding the subtract
        cin = L["cin"]
        wd_p = np.zeros((cin, W.shape[1]), f)
        wc_p = np.zeros((cin, W.shape[1]), f)
        wd_p[:c] = wd
        wc_p[:c] = wcn
        shared[f"wd{li}"] = wd_p
        shared[f"wc{li}"] = wc_p
        b = np.asarray(b, f)
        if b.shape[0] > 128:
            b = b.reshape(-1, 128).T.copy()
        else:
            b = b.reshape(-1, 1)
        shared[f"b{li}"] = np.ascontiguousarray(b)

    W5g = np.asarray(W5, f) * np.asarray(g5, f)[None, :]
    for t, (lo, hi) in zip("abcde", ((0, 64), (64, 128), (128, 256),
                                     (256, 384), (384, 512))):
        shared[f"w5{t}"] = np.ascontiguousarray(W5g[lo:hi])
    shared["b5"] = np.ascontiguousarray(np.asarray(b5, f).reshape(8, 128).T)

    pts = np.asarray(pts, f)
    in_maps = []
    for core in range(NCORES):
        x0 = np.zeros((16, N), f)
        x0[0:3] = pts[core, :, 0:3].T
        in_maps.append({"x0": x0, **shared})
    return in_maps


def kernel(**inputs):
    global LAST_RESULTS
    from concourse.bass_utils import run_bass_kernel_spmd

    nc = _get_program()
    in_maps = make_in_maps(**inputs)
    trace = bool(_CACHE.get("trace"))
    res = run_bass_kernel_spmd(nc, in_maps, list(range(NCORES)), trace=trace)
    LAST_RESULTS = res
    return np.stack([r["out"] for r in res.results]).astype(np.float32)
